# revision 12
# baseline (speedup 1.0000x reference)
"""TRN2 Bass kernel for nn_Convolution_2d: 3x3 same-padding conv2d.

X (32,128,64,64) f32  *  W (256,128,3,3)  + bias (256,)  ->  (32,256,64,64)

Strategy: data-parallel over batch across 8 NeuronCores (4 images/core).
Per core the conv is 9 accumulated matmuls per output tile with contraction
over in_ch=128 (exactly the partition dim):

    psum[o_half, pix] += W[:, ky, kx, o_half].T @ Xpad[:, y+ky, x+kx(pix)]

Matmuls run in float32r (reduced-precision fp32, full PE rate at N=512;
hardware rounds internally so raw fp32 bits are DMA'd straight into
f32r-typed tiles). PSUM is evicted with a fused bias-add on DVE.

Host side pads X (pad=1), transposes to channel-major, shards by batch,
and transposes the per-core [256,4,64,64] outputs back at the end.
"""
import numpy as np
from contextlib import ExitStack

import jax
import concourse.bass as bass
import concourse.tile as tile
from concourse import bacc, mybir
from concourse.bass2jax import (
    _bass_exec_p,
    install_neuronx_cc_hook,
    partition_id_tensor,
)
from jax.sharding import Mesh, PartitionSpec
from jax.experimental.shard_map import shard_map

N_CORES = 8
B, CIN, H, W = 32, 128, 64, 64
COUT = 256
KH = KW = 3
PAD = 1
HP, WP = H + 2 * PAD, W + 2 * PAD   # 66, 66
BC = B // N_CORES                   # images per core = 4
ROWS = 8                            # output rows per matmul tile (N = ROWS*W = 512)
M_TILES = COUT // 128               # 2

f32 = mybir.dt.float32
f32r = mybir.dt.float32r


def _build_module():
    nc = bacc.Bacc("TRN2", target_bir_lowering=False, debug=False,
                   num_devices=N_CORES)
    Xp = nc.declare_dram_parameter("Xp", [CIN, BC, HP, WP], f32r, isOutput=False)
    Wt = nc.declare_dram_parameter("Wt", [CIN, KH * KW, COUT], f32r, isOutput=False)
    bias2 = nc.declare_dram_parameter("bias2", [128, M_TILES], f32, isOutput=False)
    # output layout matches the eviction tile exactly: per partition (o) one
    # 2KB-contiguous run per DMA instead of 8x256B rows
    out = nc.declare_dram_parameter("out", [COUT, BC, H // ROWS, ROWS, W], f32,
                                    isOutput=True)

    IL = 2  # row-tile groups sharing each weight load (PSUM banks: 2*IL)

    with ExitStack() as ctx:
        tc = ctx.enter_context(tile.TileContext(nc))
        const = ctx.enter_context(tc.tile_pool(name="const", bufs=1))
        xpool = ctx.enter_context(tc.tile_pool(name="x", bufs=1))
        opool = ctx.enter_context(tc.tile_pool(name="o", bufs=4))
        psum = ctx.enter_context(tc.tile_pool(name="psum", bufs=4 * IL, space="PSUM"))

        # startup order on the SP ring: first X rows the first group needs,
        # then weights per-tap (first group consumes them at ~0.8us/tap),
        # then the rest of X
        x_sb = xpool.tile([CIN, BC, HP, WP], f32r)
        w_sb = const.tile([CIN, KH * KW, COUT], f32r)
        b_sb = const.tile([128, M_TILES], f32)

        nc.sync.dma_start(x_sb[:, 0, 0:18], Xp[:, 0, 0:18])
        for t in range(KH * KW):
            nc.sync.dma_start(w_sb[:, t], Wt[:, t])
        nc.sync.dma_start(b_sb[:], bias2[:, :])
        for r0, r1 in ((18, 34), (34, 50), (50, HP)):
            nc.sync.dma_start(x_sb[:, 0, r0:r1], Xp[:, 0, r0:r1])
        for b in range(1, BC):
            nc.sync.dma_start(x_sb[:, b], Xp[:, b])

        # warm the PE clock gate (HAM) during the initial DMA wait: dummy
        # matmuls on zeros with no input deps keep TensorE busy so the real
        # work starts at 2.4GHz instead of ramping through the cold window
        warm_f = const.tile([128, 128], f32)
        nc.vector.memset(warm_f[:], 0.0)
        warm_x = const.tile([128, 128], f32r)
        nc.vector.tensor_copy(warm_x[:], warm_f[:])
        wps = psum.tile([128, 64], f32, tag="ps", name="warm_ps")
        for _ in range(20):
            nc.tensor.matmul(wps[:], warm_x[:], warm_x[:, 0:64],
                             start=True, stop=True)

        n_y = H // ROWS
        for b in range(BC):
            for m in range(M_TILES):
                for yq in range(0, n_y, IL):
                    y0s = [(yq + j) * ROWS for j in range(IL)]
                    pss = [psum.tile([128, ROWS * W], f32, tag="ps", name=f"ps{g}")
                           for g in range(IL)]
                    for t in range(KH * KW):
                        ky, kx = t // KW, t % KW
                        lhsT = w_sb[:, t, m * 128: (m + 1) * 128]
                        for g, y0 in enumerate(y0s):
                            rhs = x_sb[:, b, y0 + ky: y0 + ky + ROWS, kx: kx + W]
                            nc.tensor.matmul(pss[g][:], lhsT, rhs,
                                             start=(t == 0), stop=(t == KH * KW - 1))
                    for g, y0 in enumerate(y0s):
                        o_sb = opool.tile([128, ROWS * W], f32, name="o_sb")
                        nc.vector.tensor_scalar_add(o_sb[:], pss[g][:],
                                                    b_sb[:, m: m + 1])
                        nc.scalar.dma_start(
                            out[m * 128: (m + 1) * 128, b, yq + g], o_sb[:])

    nc.compile()
    return nc


_CACHE = {}


def _get_runner():
    if "run" in _CACHE:
        return _CACHE["run"]

    install_neuronx_cc_hook()
    nc = _build_module()

    partition_name = nc.partition_id_tensor.name if nc.partition_id_tensor else None
    in_names, out_names, out_avals = [], [], []
    for alloc in nc.m.functions[0].allocations:
        if not isinstance(alloc, mybir.MemoryLocationSet):
            continue
        name = alloc.memorylocations[0].name
        if alloc.kind == "ExternalInput":
            if name != partition_name:
                in_names.append(name)
        elif alloc.kind == "ExternalOutput":
            out_names.append(name)
            out_avals.append(jax.core.ShapedArray(
                tuple(alloc.tensor_shape), mybir.dt.np(alloc.dtype)))
    n_params = len(in_names)
    all_in_names = list(in_names) + list(out_names)
    if partition_name is not None:
        all_in_names.append(partition_name)
    donate = tuple(range(n_params, n_params + len(out_names)))

    def _body(*args):
        operands = list(args)
        if partition_name is not None:
            operands.append(partition_id_tensor())
        return tuple(_bass_exec_p.bind(
            *operands,
            out_avals=tuple(out_avals),
            in_names=tuple(all_in_names),
            out_names=tuple(out_names),
            lowering_input_output_aliases=(),
            sim_require_finite=True,
            sim_require_nnan=True,
            nc=nc,
        ))

    devices = jax.devices()[:N_CORES]
    mesh = Mesh(np.asarray(devices), ("core",))
    n_io = n_params + len(out_names)
    jitted = jax.jit(
        shard_map(_body, mesh=mesh,
                  in_specs=(PartitionSpec("core"),) * n_io,
                  out_specs=(PartitionSpec("core"),) * len(out_names),
                  check_rep=False),
        donate_argnums=donate,
        keep_unused=True,
    )

    def run(per_core_inputs):
        concat_in = [
            np.concatenate([per_core_inputs[c][name] for c in range(N_CORES)], axis=0)
            for name in in_names
        ]
        concat_zeros = [
            np.zeros((N_CORES * a.shape[0], *a.shape[1:]), a.dtype) for a in out_avals
        ]
        out_arrs = jitted(*concat_in, *concat_zeros)
        jax.block_until_ready(out_arrs)
        return [
            {name: np.asarray(out_arrs[i]).reshape(N_CORES, *out_avals[i].shape)[c]
             for i, name in enumerate(out_names)}
            for c in range(N_CORES)
        ]

    _CACHE["run"] = run
    return run


def _prepare_inputs(X, weights, biases):
    # pad=1 both spatial dims, then channel-major (i, b, y, x)
    Xpad = np.pad(X, ((0, 0), (0, 0), (PAD, PAD), (PAD, PAD)))
    Xt = np.ascontiguousarray(Xpad.transpose(1, 0, 2, 3))      # [CIN, B, HP, WP]
    Wt = np.ascontiguousarray(
        weights.transpose(1, 2, 3, 0).reshape(CIN, KH * KW, COUT))
    bias2 = np.ascontiguousarray(biases.reshape(M_TILES, 128).T)
    per_core = []
    for c in range(N_CORES):
        per_core.append({
            "Xp": np.ascontiguousarray(Xt[:, c * BC: (c + 1) * BC]),
            "Wt": Wt,
            "bias2": bias2,
        })
    return per_core


def kernel(X, weights, biases):
    X = np.asarray(X, dtype=np.float32)
    weights = np.asarray(weights, dtype=np.float32)
    biases = np.asarray(biases, dtype=np.float32)

    run = _get_runner()
    results = run(_prepare_inputs(X, weights, biases))

    out = np.empty((B, COUT, H, W), dtype=np.float32)
    for c in range(N_CORES):
        # [COUT, BC, H//ROWS, ROWS, W] -> [BC, COUT, H, W]
        o = results[c]["out"].transpose(1, 0, 2, 3, 4).reshape(BC, COUT, H, W)
        out[c * BC: (c + 1) * BC] = o
    return out
